# revision 3
# baseline (speedup 1.0000x reference)
"""Trainium2 Bass kernel for nn_MixedDecoder (moe_routing).

Math (matches the reference exactly): only the LAST expert layer matters —
the reference never feeds layer outputs back into `z`, so layers 0/1 are
dead code.  Computed per sample b:
    coef = softmax(gate_mlp(z))                       # [B, 8]
    out  = sum_e coef[b,e] * (z @ w2[e]) + coef @ b2   # [B, 256]

Sharding: data-parallel over batch B=2048 across 8 cores (256 rows/core),
weights replicated.  Per core, the expert contraction is done as
fp32r (rounded-fp32 PE datapath ~1.6e-4 scale-relative error, full rate)
matmuls with zT chunks stationary and expert pairs side-by-side as a
[K,512] moving operand; per-expert coefficient scaling happens on
PSUM eviction (per-partition scalar), followed by a pairwise add tree.
"""

import numpy as np

N_CORES = 8
B = 2048
IN_SIZE = 288
HIDDEN = 256
E = 8
GATE_H = 64
OUT_SIZE = 256
BL = B // N_CORES          # 256 rows per core
NCH = BL // 128            # 2 batch chunks of 128
KB = [(0, 128), (128, 128), (256, 32)]   # K chunks of IN_SIZE=288

_CACHE = {}


def _build_nc(reps=1):
    from concourse import bacc
    import concourse.mybir as mybir
    from concourse.tile import TileContext
    from concourse.masks import make_identity

    dt = mybir.dt
    F32 = dt.float32
    F32R = dt.float32r
    AF = mybir.ActivationFunctionType
    OP = mybir.AluOpType
    AX = mybir.AxisListType

    nc = bacc.Bacc("TRN2", target_bir_lowering=False, debug=False)

    z_d = nc.declare_dram_parameter("z", [BL, IN_SIZE], F32, isOutput=False)
    g0w_d = nc.declare_dram_parameter("g0_w", [IN_SIZE, GATE_H], F32, isOutput=False)
    g0b_d = nc.declare_dram_parameter("g0_b", [GATE_H, 1], F32, isOutput=False)
    g1w_d = nc.declare_dram_parameter("g1_w", [GATE_H, GATE_H], F32, isOutput=False)
    g1b_d = nc.declare_dram_parameter("g1_b", [GATE_H, 1], F32, isOutput=False)
    g2w_d = nc.declare_dram_parameter("g2_w", [GATE_H, E], F32, isOutput=False)
    g2b_d = nc.declare_dram_parameter("g2_b", [1, E], F32, isOutput=False)
    w2_d = nc.declare_dram_parameter("w2", [E, IN_SIZE, OUT_SIZE], F32, isOutput=False)
    b2_d = nc.declare_dram_parameter("b2", [E, OUT_SIZE], F32, isOutput=False)
    out_d = nc.declare_dram_parameter("out", [BL, OUT_SIZE], F32, isOutput=True)

    with TileContext(nc) as tc:
      for _rep in range(reps):
        with (
            tc.tile_pool(name="const", bufs=1) as cp,
            tc.tile_pool(name="w2p", bufs=1) as wp,
            tc.tile_pool(name="ztp", bufs=1) as zp,
            tc.tile_pool(name="wk", bufs=2) as wk,
            tc.tile_pool(name="py", bufs=4, space="PSUM") as py,
            tc.tile_pool(name="pb", bufs=2, space="PSUM") as pb,
            tc.tile_pool(name="pg", bufs=2, space="PSUM") as pg,
        ):
            # ---------------- constants / weights ----------------
            ident = cp.tile([128, 128], F32, name="ident")
            make_identity(nc, ident[:])

            # z chunks
            z_sb = []
            for c in range(NCH):
                t = cp.tile([128, IN_SIZE], F32, name=f"z{c}")
                nc.sync.dma_start(out=t[:], in_=z_d.ap()[c * 128:(c + 1) * 128, :])
                z_sb.append(t)

            # gate weights (fp32 originals where needed + fp32r rounded)
            g0w_r = []
            for i, (k0, ksz) in enumerate(KB):
                t = cp.tile([ksz, GATE_H], F32, name=f"g0w{i}")
                nc.sync.dma_start(out=t[:], in_=g0w_d.ap()[k0:k0 + ksz, :])
                tr = cp.tile([ksz, GATE_H], F32R, name=f"g0wr{i}")
                nc.vector.tensor_copy(tr[:], t[:])
                g0w_r.append(tr)
            g1w = cp.tile([GATE_H, GATE_H], F32, name="g1w")
            nc.sync.dma_start(out=g1w[:], in_=g1w_d.ap())
            g1w_r = cp.tile([GATE_H, GATE_H], F32R, name="g1wr")
            nc.vector.tensor_copy(g1w_r[:], g1w[:])
            g2w = cp.tile([GATE_H, E], F32, name="g2w")
            nc.sync.dma_start(out=g2w[:], in_=g2w_d.ap())
            g2w_r = cp.tile([GATE_H, E], F32R, name="g2wr")
            nc.vector.tensor_copy(g2w_r[:], g2w[:])

            g0b = cp.tile([GATE_H, 1], F32, name="g0b")
            nc.sync.dma_start(out=g0b[:], in_=g0b_d.ap())
            g0bn = cp.tile([GATE_H, 1], F32, name="g0bn")
            nc.vector.tensor_scalar(g0bn[:], g0b[:], -1.0, None, OP.mult)
            g1b = cp.tile([GATE_H, 1], F32, name="g1b")
            nc.sync.dma_start(out=g1b[:], in_=g1b_d.ap())
            g1bn = cp.tile([GATE_H, 1], F32, name="g1bn")
            nc.vector.tensor_scalar(g1bn[:], g1b[:], -1.0, None, OP.mult)
            g2b_row = cp.tile([1, E], F32, name="g2b_row")
            nc.sync.dma_start(out=g2b_row[:], in_=g2b_d.ap())
            ones_row = cp.tile([1, 128], F32, name="ones_row")
            nc.vector.memset(ones_row[:], 1.0)

            b2_sb = cp.tile([E, OUT_SIZE], F32, name="b2")
            nc.sync.dma_start(out=b2_sb[:], in_=b2_d.ap())
            b2_r = cp.tile([E, OUT_SIZE], F32R, name="b2r")
            nc.vector.tensor_copy(b2_r[:], b2_sb[:])

            # w2 expert-pair tiles: [ksz, 512] = [w2[2p] | w2[2p+1]] per K chunk
            w2_r = {}
            rnd_i = 0
            for i, (k0, ksz) in enumerate(KB):
                for p in range(E // 2):
                    t = wp.tile([ksz, 2 * OUT_SIZE], F32, name=f"w2_{i}_{p}")
                    nc.sync.dma_start(out=t[:, 0:OUT_SIZE],
                                      in_=w2_d.ap()[2 * p, k0:k0 + ksz, :])
                    nc.sync.dma_start(out=t[:, OUT_SIZE:2 * OUT_SIZE],
                                      in_=w2_d.ap()[2 * p + 1, k0:k0 + ksz, :])
                    tr = wp.tile([ksz, 2 * OUT_SIZE], F32R, name=f"w2r_{i}_{p}")
                    if rnd_i % 2 == 0:
                        nc.vector.tensor_copy(tr[:], t[:])
                    else:
                        nc.scalar.copy(tr[:], t[:])
                    rnd_i += 1
                    w2_r[(i, p)] = tr

            # ---------------- zT via PE transpose ----------------
            zT_r = []
            for i, (k0, ksz) in enumerate(KB):
                t = zp.tile([ksz, BL], F32R, name=f"zT{i}")
                zT_r.append(t)
            for c in range(NCH):
                for i, (k0, ksz) in enumerate(KB):
                    tp = pg.tile([ksz, 128], F32, name="ztp", tag="pg")
                    nc.tensor.transpose(tp[:], z_sb[c][:, k0:k0 + ksz], ident[:])
                    nc.vector.tensor_copy(zT_r[i][:, c * 128:(c + 1) * 128], tp[:])

            # ---------------- gate MLP (transposed layout) ----------------
            def elu_t(ps_in, bias, bias_neg, out_r, pref):
                """out_r = fp32r( elu(ps_in + bias) ) for [64, 256] tiles."""
                t_relu = wk.tile([GATE_H, BL], F32, name=f"{pref}_relu")
                nc.scalar.activation(t_relu[:], ps_in, AF.Relu, bias=bias[:])
                t_nmin = wk.tile([GATE_H, BL], F32, name=f"{pref}_nmin")
                nc.scalar.activation(t_nmin[:], ps_in, AF.Relu,
                                     bias=bias_neg[:], scale=-1.0)
                t_exp = wk.tile([GATE_H, BL], F32, name=f"{pref}_exp")
                nc.scalar.activation(t_exp[:], t_nmin[:], AF.Exp, scale=-1.0)
                t_sum = wk.tile([GATE_H, BL], F32, name=f"{pref}_sum")
                nc.vector.tensor_tensor(t_sum[:], t_relu[:], t_exp[:], OP.add)
                nc.vector.tensor_scalar(out_r[:], t_sum[:], -1.0, None, OP.add)

            h0_ps = pg.tile([GATE_H, BL], F32, name="h0ps", tag="pg")
            for i in range(3):
                nc.tensor.matmul(h0_ps[:], g0w_r[i][:], zT_r[i][:],
                                 start=(i == 0), stop=(i == 2))
            h0_r = wk.tile([GATE_H, BL], F32R, name="h0r")
            elu_t(h0_ps[:], g0b, g0bn, h0_r, "e0")

            h1_ps = pg.tile([GATE_H, BL], F32, name="h1ps", tag="pg")
            nc.tensor.matmul(h1_ps[:], g1w_r[:], h0_r[:], start=True, stop=True)
            h1_r = wk.tile([GATE_H, BL], F32R, name="h1r")
            elu_t(h1_ps[:], g1b, g1bn, h1_r, "e1")

            # logits in [b, 8] layout per chunk + g2_b broadcast-accumulate
            coef_sb = []     # normalized coefficients [128, 8] fp32, per chunk
            coefT_r = zp.tile([E, BL], F32R, name="coefT")
            for c in range(NCH):
                lg_ps = pg.tile([128, E], F32, name="lgps", tag="pg")
                nc.tensor.matmul(lg_ps[:], h1_r[:, c * 128:(c + 1) * 128],
                                 g2w_r[:], start=True, stop=False)
                nc.tensor.matmul(lg_ps[:], ones_row[:], g2b_row[:],
                                 start=False, stop=True)
                nmax = wk.tile([128, 1], F32, name="nmax")
                nc.vector.tensor_reduce(nmax[:], lg_ps[:], AX.X, OP.max,
                                        negate=True)
                expc = wk.tile([128, E], F32, name="expc")
                sume = wk.tile([128, 1], F32, name="sume")
                nc.scalar.activation(expc[:], lg_ps[:], AF.Exp, bias=nmax[:],
                                     accum_out=sume[:])
                rcp = wk.tile([128, 1], F32, name="rcp")
                nc.vector.reciprocal(rcp[:], sume[:])
                cf = wk.tile([128, E], F32, name="coef")
                nc.vector.tensor_scalar(cf[:], expc[:], rcp[:], None, OP.mult)
                coef_sb.append(cf)
                ctp = pg.tile([E, 128], F32, name="ctp", tag="pg")
                nc.tensor.transpose(ctp[:], cf[:], ident[:])
                nc.vector.tensor_copy(coefT_r[:, c * 128:(c + 1) * 128], ctp[:])

            # ---------------- expert layer + combine ----------------
            for c in range(NCH):
                ys = []
                for p in range(E // 2):
                    yp = py.tile([128, 2 * OUT_SIZE], F32, name=f"yp{p}", tag="py")
                    for i in range(3):
                        nc.tensor.matmul(
                            yp[:], zT_r[i][:, c * 128:(c + 1) * 128],
                            w2_r[(i, p)][:], start=(i == 0), stop=(i == 2))
                    for h in range(2):
                        e = 2 * p + h
                        t = wk.tile([128, OUT_SIZE], F32, name=f"ys{e}")
                        src = yp[:, h * OUT_SIZE:(h + 1) * OUT_SIZE]
                        scale = coef_sb[c][:, e:e + 1]
                        if e % 2 == 0:
                            nc.scalar.activation(t[:], src, AF.Copy, scale=scale)
                        else:
                            nc.vector.tensor_scalar(t[:], src, scale, None, OP.mult)
                        ys.append(t)

                # mixed bias:  coefT[:, chunk].T @ b2  -> [128, 256]
                bias_ps = pb.tile([128, OUT_SIZE], F32, name="biasps", tag="pb")
                nc.tensor.matmul(bias_ps[:], coefT_r[:, c * 128:(c + 1) * 128],
                                 b2_r[:], start=True, stop=True)

                # pairwise add tree on DVE
                lvl = ys
                li = 0
                while len(lvl) > 1:
                    nxt = []
                    for j in range(0, len(lvl), 2):
                        t = wk.tile([128, OUT_SIZE], F32, name=f"tr{li}_{j}")
                        nc.vector.tensor_tensor(t[:], lvl[j][:], lvl[j + 1][:],
                                                OP.add)
                        nxt.append(t)
                    lvl = nxt
                    li += 1
                out_sb = wk.tile([128, OUT_SIZE], F32, name="outsb")
                nc.vector.tensor_tensor(out_sb[:], lvl[0][:], bias_ps[:], OP.add)
                nc.sync.dma_start(out=out_d.ap()[c * 128:(c + 1) * 128, :],
                                  in_=out_sb[:])

    nc.finalize()
    return nc


def _get_nc(reps=1):
    key = ("nc", reps)
    if key not in _CACHE:
        _CACHE[key] = _build_nc(reps)
    return _CACHE[key]


def make_in_maps(z, g0_w, g0_b, g1_w, g1_b, g2_w, g2_b, w2, b2, **_unused):
    f = np.ascontiguousarray
    shared = {
        "g0_w": f(g0_w, dtype=np.float32),
        "g0_b": f(g0_b, dtype=np.float32).reshape(GATE_H, 1),
        "g1_w": f(g1_w, dtype=np.float32),
        "g1_b": f(g1_b, dtype=np.float32).reshape(GATE_H, 1),
        "g2_w": f(g2_w, dtype=np.float32),
        "g2_b": f(g2_b, dtype=np.float32).reshape(1, E),
        "w2": f(w2, dtype=np.float32),
        "b2": f(b2, dtype=np.float32),
    }
    z = f(z, dtype=np.float32)
    return [dict(shared, z=z[c * BL:(c + 1) * BL]) for c in range(N_CORES)]


def kernel(**inputs):
    from concourse.bass_utils import run_bass_kernel_spmd

    nc = _get_nc()
    in_maps = make_in_maps(**inputs)
    res = run_bass_kernel_spmd(nc, in_maps, list(range(N_CORES)))
    return np.concatenate(
        [res.results[c]["out"] for c in range(N_CORES)], axis=0
    ).astype(np.float32)


# revision 15
# speedup vs baseline: 12.4642x; 12.4642x over previous
"""Trainium2 Bass kernel for nn_MixedDecoder (moe_routing).

Math (matches the reference exactly): only the LAST expert layer matters —
the reference never feeds layer outputs back into `z`, so layers 0/1 are
dead code.  Computed per sample b:
    coef = softmax(gate_mlp(z))                        # [B, 8]
    out  = sum_e coef[b,e] * (z @ w2[e]) + coef @ b2   # [B, 256]

Sharding: data-parallel over batch B=2048 across 8 cores (256 rows/core),
weights replicated.  Host-side numpy packs inputs (including a
pre-transposed z) so each core does 8 input DMAs + 1 output DMA and no
on-chip transposes of z.  Matmul inputs are float32r DRAM parameters
(rounded-fp32 PE datapath: bf16 rate at N>=256, ~1.6e-4 scale-relative
error).  ELU is computed as relu(x)+min(exp(x),1) (monotonicity folds the
min into the exp) with the "+1" offset folded into adjusted next-layer
biases.  Expert matmuls keep zT chunks stationary with expert pairs
side-by-side as a [K,512] moving operand; per-expert coefficient scaling
happens on PSUM eviction (per-partition scalar), then a pairwise add tree
split across DVE and GPSIMD.
"""

import numpy as np

N_CORES = 8
B = 2048
IN_SIZE = 288
HIDDEN = 256
E = 8
GATE_H = 64
OUT_SIZE = 256
BL = B // N_CORES          # 256 rows per core
NCH = BL // 128            # 2 batch chunks of 128
KC = 96                    # K chunk size (288 = 3 x 96)
NK = IN_SIZE // KC
W = E * OUT_SIZE           # 2048: one K-chunk's width of packed w2

_CACHE = {}


def _build_nc(reps=1):
    from concourse import bacc
    import concourse.mybir as mybir
    from concourse.tile import TileContext
    from concourse.masks import make_identity

    dt = mybir.dt
    F32 = dt.float32
    F32R = dt.float32r
    AF = mybir.ActivationFunctionType
    OP = mybir.AluOpType
    AX = mybir.AxisListType

    nc = bacc.Bacc("TRN2", target_bir_lowering=False, debug=False)

    # packed inputs (see make_in_maps)
    zT_d = nc.declare_dram_parameter("zTp", [KC, NK * BL], F32R, isOutput=False)
    g0w_d = nc.declare_dram_parameter("g0wp", [KC, NK * GATE_H], F32R, isOutput=False)
    gw12_d = nc.declare_dram_parameter("gw12", [GATE_H, GATE_H + E], F32R,
                                       isOutput=False)
    # biases pack: col0 g0_b | col1 g1_b | row0 cols 2:10 g2_b
    sm_d = nc.declare_dram_parameter("smallp", [GATE_H, 10], F32, isOutput=False)
    b2_d = nc.declare_dram_parameter("b2", [E, OUT_SIZE], F32R, isOutput=False)
    # w2 packed: [96, 3*2048]; chunk i cols = w2.transpose(1,0,2)[i*96:(i+1)*96]
    w2_d = nc.declare_dram_parameter("w2p", [KC, NK * W], F32R, isOutput=False)
    out_d = nc.declare_dram_parameter("outp", [128, NCH * OUT_SIZE], F32,
                                      isOutput=True)

    with TileContext(nc) as tc:
      for _rep in range(reps):
        with (
            tc.tile_pool(name="const", bufs=1) as cp,
            tc.tile_pool(name="w2p", bufs=1) as wp,
            tc.tile_pool(name="wk", bufs=2) as wk,
            tc.tile_pool(name="py", bufs=4, space="PSUM") as py,
            tc.tile_pool(name="pb", bufs=2, space="PSUM") as pb,
            tc.tile_pool(name="pg", bufs=2, space="PSUM") as pg,
        ):
            # -------- DMAs: gate inputs first, then w2 chunks, then b2 ------
            zT_r = cp.tile([KC, NK * BL], F32R, name="zT")
            nc.sync.dma_start(out=zT_r[:], in_=zT_d.ap())
            g0w_r = cp.tile([KC, NK * GATE_H], F32R, name="g0wr")
            nc.sync.dma_start(out=g0w_r[:], in_=g0w_d.ap())
            gw12_r = cp.tile([GATE_H, GATE_H + E], F32R, name="gw12r")
            nc.sync.dma_start(out=gw12_r[:], in_=gw12_d.ap())
            sm = cp.tile([GATE_H, 10], F32, name="sm")
            nc.sync.dma_start(out=sm[:], in_=sm_d.ap())
            w2_r = wp.tile([KC, NK * W], F32R, name="w2r")
            for i in range(NK):
                nc.sync.dma_start(out=w2_r[:, i * W:(i + 1) * W],
                                  in_=w2_d.ap()[:, i * W:(i + 1) * W])
            b2_r = cp.tile([E, OUT_SIZE], F32R, name="b2r")
            nc.sync.dma_start(out=b2_r[:], in_=b2_d.ap())

            ident = cp.tile([128, 128], F32, name="ident")
            make_identity(nc, ident[:])

            # dummy exp so the ACT Exp-table load happens before it's needed
            warm = cp.tile([1, 1], F32, name="warm")
            nc.vector.memset(warm[:], 0.0)
            warm2 = cp.tile([1, 1], F32, name="warm2")
            nc.scalar.activation(warm2[:], warm[:], AF.Exp)

            g1w_r = gw12_r[:, 0:GATE_H]
            g2w_r = gw12_r[:, GATE_H:GATE_H + E]
            g0b = sm[:, 0:1]
            b1_adj = sm[:, 1:2]          # g1_b - colsum(g1_w), host-computed
            adj2_row = sm[0:1, 2:2 + E]  # g2_b - colsum(g2_w), host-computed
            ones_row = cp.tile([1, 128], F32, name="ones_row")
            nc.vector.memset(ones_row[:], 1.0)

            # ---------------- gate MLP (transposed layout) ----------------
            def elu_p1(ps_in, bias, out_r, pref):
                """out_r = fp32r( elu(ps_in + bias) + 1 ) for [64, 256] tiles.

                elu(x)+1 = relu(x) + min(exp(x), 1); exp on ACT, rest on DVE.
                """
                t_exp = wk.tile([GATE_H, BL], F32, name=f"{pref}_exp")
                nc.scalar.activation(t_exp[:], ps_in, AF.Exp, bias=bias)
                t_relu = wk.tile([GATE_H, BL], F32, name=f"{pref}_relu")
                nc.vector.tensor_scalar(t_relu[:], ps_in, bias, 0.0, OP.add, OP.max)
                t_min = wk.tile([GATE_H, BL], F32, name=f"{pref}_min")
                nc.vector.tensor_scalar(t_min[:], t_exp[:], 1.0, None, OP.min)
                nc.vector.tensor_tensor(out_r[:], t_relu[:], t_min[:], OP.add)

            h0_ps = pg.tile([GATE_H, BL], F32, name="h0ps", tag="pg")
            for i in range(NK):
                nc.tensor.matmul(h0_ps[:], g0w_r[:, i * GATE_H:(i + 1) * GATE_H],
                                 zT_r[:, i * BL:(i + 1) * BL],
                                 start=(i == 0), stop=(i == NK - 1))
            h0_r = wk.tile([GATE_H, BL], F32R, name="h0r")
            elu_p1(h0_ps[:], g0b, h0_r, "e0")

            h1_ps = pg.tile([GATE_H, BL], F32, name="h1ps", tag="pg")
            nc.tensor.matmul(h1_ps[:], g1w_r, h0_r[:], start=True, stop=True)
            h1_r = wk.tile([GATE_H, BL], F32R, name="h1r")
            elu_p1(h1_ps[:], b1_adj, h1_r, "e1")

            # logits in [b, 8] layout per chunk + adjusted-bias broadcast
            coef_sb = []     # normalized coefficients [128, 8] fp32, per chunk
            coefT_r = cp.tile([E, BL], F32R, name="coefT")
            for c in range(NCH):
                lg_ps = pg.tile([128, E], F32, name="lgps", tag="pg")
                nc.tensor.matmul(lg_ps[:], h1_r[:, c * 128:(c + 1) * 128],
                                 g2w_r, start=True, stop=False)
                nc.tensor.matmul(lg_ps[:], ones_row[:], adj2_row,
                                 start=False, stop=True)
                expc = wk.tile([128, E], F32, name="expc")
                sume = wk.tile([128, 1], F32, name="sume")
                nc.scalar.activation(expc[:], lg_ps[:], AF.Exp,
                                     accum_out=sume[:])
                rcp = wk.tile([128, 1], F32, name="rcp")
                nc.vector.reciprocal(rcp[:], sume[:])
                cf = wk.tile([128, E], F32, name="coef")
                nc.vector.tensor_scalar(cf[:], expc[:], rcp[:], None, OP.mult)
                coef_sb.append(cf)
                ctp = pg.tile([E, 128], F32, name="ctp", tag="pg")
                nc.tensor.transpose(ctp[:], cf[:], ident[:])
                nc.vector.tensor_copy(coefT_r[:, c * 128:(c + 1) * 128], ctp[:])

            # ---------------- expert layer + combine ----------------
            out_sb = wk.tile([128, NCH * OUT_SIZE], F32, name="outsb")
            for c in range(NCH):
                ys = []
                for p in range(E // 2):
                    yp = py.tile([128, 2 * OUT_SIZE], F32, name=f"yp{p}", tag="py")
                    for i in range(NK):
                        col0 = i * W + 2 * p * OUT_SIZE
                        nc.tensor.matmul(
                            yp[:], zT_r[:, i * BL + c * 128:i * BL + (c + 1) * 128],
                            w2_r[:, col0:col0 + 2 * OUT_SIZE],
                            start=(i == 0), stop=(i == NK - 1))
                    for h in range(2):
                        e = 2 * p + h
                        t = wk.tile([128, OUT_SIZE], F32, name=f"ys{e}")
                        src = yp[:, h * OUT_SIZE:(h + 1) * OUT_SIZE]
                        scale = coef_sb[c][:, e:e + 1]
                        if h == 0:
                            nc.scalar.activation(t[:], src, AF.Copy, scale=scale)
                        else:
                            nc.vector.tensor_scalar(t[:], src, scale, None, OP.mult)
                        ys.append(t)

                # mixed bias:  coefT[:, chunk].T @ b2  -> [128, 256]
                bias_ps = pb.tile([128, OUT_SIZE], F32, name="biasps", tag="pb")
                nc.tensor.matmul(bias_ps[:], coefT_r[:, c * 128:(c + 1) * 128],
                                 b2_r[:], start=True, stop=True)

                # add tree: L1 split DVE/GPSIMD, rest DVE
                l1 = []
                for j in range(4):
                    t = wk.tile([128, OUT_SIZE], F32, name=f"t1_{j}")
                    eng = nc.vector if j % 2 == 0 else nc.gpsimd
                    eng.tensor_tensor(t[:], ys[2 * j][:], ys[2 * j + 1][:], OP.add)
                    l1.append(t)
                q0 = wk.tile([128, OUT_SIZE], F32, name="q0")
                nc.vector.tensor_tensor(q0[:], l1[0][:], l1[1][:], OP.add)
                q1 = wk.tile([128, OUT_SIZE], F32, name="q1")
                nc.gpsimd.tensor_tensor(q1[:], l1[2][:], l1[3][:], OP.add)
                hsum = wk.tile([128, OUT_SIZE], F32, name="hsum")
                nc.vector.tensor_tensor(hsum[:], q0[:], q1[:], OP.add)
                nc.vector.tensor_tensor(out_sb[:, c * OUT_SIZE:(c + 1) * OUT_SIZE],
                                        hsum[:], bias_ps[:], OP.add)
            nc.sync.dma_start(out=out_d.ap(), in_=out_sb[:])

    nc.finalize()
    return nc


def _get_nc(reps=1):
    key = ("nc", reps)
    if key not in _CACHE:
        _CACHE[key] = _build_nc(reps)
    return _CACHE[key]


def make_in_maps(z, g0_w, g0_b, g1_w, g1_b, g2_w, g2_b, w2, b2, **_unused):
    z = np.asarray(z, dtype=np.float32)
    g0_w = np.asarray(g0_w, dtype=np.float32)
    g1_w = np.asarray(g1_w, dtype=np.float32)
    g2_w = np.asarray(g2_w, dtype=np.float32)
    g0_b = np.asarray(g0_b, dtype=np.float32)
    g1_b = np.asarray(g1_b, dtype=np.float32)
    g2_b = np.asarray(g2_b, dtype=np.float32)
    w2 = np.asarray(w2, dtype=np.float32)
    b2 = np.ascontiguousarray(b2, dtype=np.float32)

    g0wp = np.concatenate([g0_w[i * KC:(i + 1) * KC] for i in range(NK)], axis=1)
    gw12 = np.concatenate([g1_w, g2_w], axis=1)
    smallp = np.zeros((GATE_H, 10), dtype=np.float32)
    smallp[:, 0] = g0_b
    # adjusted biases absorb the ELU "+1" offset of the previous layer
    smallp[:, 1] = g1_b - g1_w.sum(axis=0)
    smallp[0, 2:2 + E] = g2_b - g2_w.sum(axis=0)
    w2t = np.ascontiguousarray(w2.transpose(1, 0, 2)).reshape(IN_SIZE, W)
    w2p = np.concatenate([w2t[i * KC:(i + 1) * KC] for i in range(NK)], axis=1)

    shared = {
        "g0wp": np.ascontiguousarray(g0wp),
        "gw12": np.ascontiguousarray(gw12),
        "smallp": smallp,
        "w2p": np.ascontiguousarray(w2p),
        "b2": b2,
    }
    maps = []
    for c in range(N_CORES):
        zT = z[c * BL:(c + 1) * BL].T                      # [288, 256]
        zTp = np.concatenate([zT[i * KC:(i + 1) * KC] for i in range(NK)],
                             axis=1)                        # [96, 768]
        maps.append(dict(shared, zTp=np.ascontiguousarray(zTp)))
    return maps


def unpack_out(res_list):
    full = np.empty((B, OUT_SIZE), dtype=np.float32)
    for c in range(N_CORES):
        packed = res_list[c]["outp"]
        for ch in range(NCH):
            full[c * BL + ch * 128:c * BL + (ch + 1) * 128] = \
                packed[:, ch * OUT_SIZE:(ch + 1) * OUT_SIZE]
    return full


def kernel(**inputs):
    from concourse.bass_utils import run_bass_kernel_spmd

    nc = _get_nc()
    in_maps = make_in_maps(**inputs)
    res = run_bass_kernel_spmd(nc, in_maps, list(range(N_CORES)))
    return unpack_out(res.results)
